# revision 1
# baseline (speedup 1.0000x reference)
"""nn_DCAttention Trainium2 kernel: full inputs -> full output, SPMD over 8 NeuronCores.

Sharding:
  Phase A (projections): token-parallel (8 blocks of 512 tokens; conv halo
  comes in with the pre-transposed input, zero-padded at batch edges).
  A2A #1 re-shards Q/K/V/tau/delta to head-pair-parallel (core c: heads 2c,2c+1).
  Phase B: attention per (batch, head), flash-style streaming over key tiles,
  all in transposed (dims, tokens) layout; softmax without max-subtraction
  (scores are bounded by construction: |raw|/8 * sigmoid + sigmoid).
  A2A #2 re-shards attention output back to token-parallel for out_proj.
All matmuls run in float32r (full PE rate at moving-dim >= 256, ~1e-4 accuracy).
"""
import numpy as np
import concourse.bass as bass
import concourse.tile as tile
import concourse.mybir as mybir
from concourse import bacc

f32 = mybir.dt.float32
f32r = mybir.dt.float32r
AF = mybir.ActivationFunctionType
ALU = mybir.AluOpType

D, H, B, L = 1024, 16, 2, 2048
DK = D // H          # 64
NCORES = 8
T = (B * L) // NCORES  # 512 tokens per core
TH = T + 2             # with halo
KT = D // 128          # 8 k-tiles for D contraction
GROUPS = [[0, 1, 2, 3, 4, 5, 6, 7]]

# A2A #1 shard layout, per head-pair shard (rows x 512):
#   rows 0:128    K^T  (128 dims, 512 tok)
#   rows 128:256  Q^T
#   rows 256:384  V natural (512 tok, 128 dims) viewed as flat
#   rows 384:388  tau'(2 heads) then delta(2 heads)
A2A1_ROWS = 388
A2A2_ROWS = 128
GELU_FUNC = AF.Gelu  # sim lacks Gelu; tests may substitute


def build(debug_outputs=(), repeat=1):
    nc = bacc.Bacc(None, target_bir_lowering=False, debug=False)
    nc.num_devices = NCORES

    dp = lambda name, shape, dtype=f32: nc.declare_dram_parameter(name, list(shape), dtype, isOutput=False)
    xT = dp("xT", (D, TH))                    # x^T with halo, zero-padded
    WqT = dp("WqT", (D, D)); Wq_b = dp("Wq_b", (D,))
    WkT = dp("WkT", (D, D)); Wk_b = dp("Wk_b", (D,))
    WvT = dp("WvT", (D, D)); Wv_b = dp("Wv_b", (D,))
    cqT = dp("cqT", (3, D, D)); cq_b = dp("cq_b", (D,))   # convq_w[:,:,k].T stacked
    ckT = dp("ckT", (3, D, D)); ck_b = dp("ck_b", (D,))
    qpT = dp("qpT", (2 * D, D)); qp_b = dp("qp_b", (D,))
    kpT = dp("kpT", (2 * D, D)); kp_b = dp("kp_b", (D,))
    tau1p = dp("tau1p", (2 * D, 4))           # [w0 w1 w2 b]
    del1p = dp("del1p", (2 * D, 4))
    tau2T = dp("tau2T", (2 * D, H)); tau2_b = dp("tau2_b", (H,))
    del2T = dp("del2T", (2 * D, H)); del2_b = dp("del2_b", (H,))
    outT = dp("outT", (D, D)); out_b = dp("out_b", (D,))
    mask_lo = dp("mask_lo", (1,))   # 0.0 when left halo is outside the batch
    mask_hi = dp("mask_hi", (1,))

    yT = nc.declare_dram_parameter("yT", [D, T], f32, isOutput=True)

    dbg = {}
    for name, shape in [
        ("k_inT", (D, TH)), ("k3T", (D, T)), ("kT_", (D, T)),
        ("q_inT", (D, TH)), ("q3T", (D, T)), ("qT_", (D, T)),
        ("V_", (T, D)), ("tau", (H, T)), ("delta", (H, T)),
        ("a2a1_out", (NCORES, A2A1_ROWS, T)), ("attnT", (D, T)),
    ]:
        if name in debug_outputs:
            dbg[name] = nc.declare_dram_parameter("dbg_" + name, list(shape), f32, isOutput=True)

    a2a1_in = nc.dram_tensor("a2a1_in", [NCORES, A2A1_ROWS, T], f32r)
    a2a1_out = nc.dram_tensor("a2a1_out", [NCORES, A2A1_ROWS, T], f32r)
    a2a2_in = nc.dram_tensor("a2a2_in", [NCORES, A2A2_ROWS, T], f32r)
    a2a2_out = nc.dram_tensor("a2a2_out", [NCORES, A2A2_ROWS, T], f32r)

    env = dict(locals())
    with tile.TileContext(nc) as tc:
        for _rep in range(repeat):
            _body(nc, tc, env)
    nc.finalize()
    return nc, dbg


def _body(nc, tc, env):
    g = lambda n: env[n]
    xT, yT, dbg = g("xT"), g("yT"), g("dbg")
    a2a1_in, a2a1_out, a2a2_in, a2a2_out = g("a2a1_in"), g("a2a1_out"), g("a2a2_in"), g("a2a2_out")

    with (
        tc.tile_pool(name="xp", bufs=1) as xp,            # x^T rounded, persistent
        tc.tile_pool(name="const", bufs=1) as constp,
        tc.tile_pool(name="wpool", bufs=2) as wpool,      # streamed weight slices
        tc.tile_pool(name="cwpool", bufs=6) as cwpool,    # conv weights (3 taps live)
        tc.tile_pool(name="vwpool", bufs=2) as vwpool,
        tc.tile_pool(name="actp", bufs=1) as actp,        # k_inT / K3T (reused for q)
        tc.tile_pool(name="evp", bufs=4) as evp,          # psum eviction tiles
        tc.tile_pool(name="tdp", bufs=3) as tdp,          # tau/delta working tiles
        tc.tile_pool(name="ps", bufs=4, space="PSUM") as ps,
        tc.tile_pool(name="ps_td", bufs=2, space="PSUM") as ps_td,
    ):
        # ---- load x^T directly as f32r ----
        xr = xp.tile([128, KT, TH], f32r, tag="xr")
        nc.sync.dma_start(out=xr[:], in_=xT.rearrange("(kt p) t -> p kt t", p=128).bitcast(f32r))

        # ---- biases (per-partition column tiles) ----
        def load_col(name, n=1024):
            t_ = constp.tile([128, n // 128], f32, tag="bias_" + name)
            nc.sync.dma_start(out=t_[:], in_=g(name).rearrange("(mt p) -> p mt", p=128))
            return t_
        b_wq, b_wk = load_col("Wq_b"), load_col("Wk_b")
        b_cq, b_ck = load_col("cq_b"), load_col("ck_b")
        b_qp, b_kp = load_col("qp_b"), load_col("kp_b")
        bv = constp.tile([128, 1024], f32, tag="bv")
        nc.sync.dma_start(out=bv[:], in_=g("Wv_b").ap().unsqueeze(0).broadcast_to([128, 1024]))
        b_tau2 = constp.tile([16, 1], f32, tag="b_tau2")
        nc.sync.dma_start(out=b_tau2[:], in_=g("tau2_b").rearrange("(p o) -> p o", o=1))
        b_del2 = constp.tile([16, 1], f32, tag="b_del2")
        nc.sync.dma_start(out=b_del2[:], in_=g("del2_b").rearrange("(p o) -> p o", o=1))
        m_lo = constp.tile([128, 1], f32, tag="m_lo")
        nc.sync.dma_start(out=m_lo[:], in_=g("mask_lo").ap().unsqueeze(0).broadcast_to([128, 1]))
        m_hi = constp.tile([128, 1], f32, tag="m_hi")
        nc.sync.dma_start(out=m_hi[:], in_=g("mask_hi").ap().unsqueeze(0).broadcast_to([128, 1]))

        def stream_w(pool, ap, cin, mt, mwidth=128, tag="w"):
            """DMA (cin, mwidth) slice for output tile mt -> (128, cin//128, mwidth) f32r."""
            wt = pool.tile([128, cin // 128, mwidth], f32r, tag=tag)
            nc.sync.dma_start(
                out=wt[:],
                in_=ap[:, mt * mwidth:(mt + 1) * mwidth]
                .rearrange("(kt p) m -> p kt m", p=128).bitcast(f32r))
            return wt

        NCH = [(0, 512), (512, 2)]  # halo-width N chunks

        def branch(WT, b_w, cT, b_c, pT, b_p, qk_row0, pref):
            """Q or K branch: linear -> conv3 -> proj; writes proj^T tiles into a2a1_in."""
            in_t = actp.tile([128, KT, TH], f32r, tag="lin")
            for mt in range(KT):
                wt = stream_w(wpool, WT, D, mt, tag="lin_w")
                for (n0, nw) in NCH:
                    p = ps.tile([128, 512], f32, tag="pA")
                    for kt in range(KT):
                        nc.tensor.matmul(p[:, :nw], wt[:, kt, :], xr[:, kt, n0:n0 + nw],
                                         start=(kt == 0), stop=(kt == KT - 1))
                    nc.vector.tensor_scalar_add(in_t[:, mt, n0:n0 + nw], p[:, :nw],
                                                b_w[:, mt:mt + 1])
                # conv zero-padding: kill halo columns outside the batch
                nc.vector.tensor_scalar(in_t[:, mt, 0:1], in_t[:, mt, 0:1],
                                        m_lo[:, 0:1], None, op0=ALU.mult)
                nc.vector.tensor_scalar(in_t[:, mt, TH - 1:TH], in_t[:, mt, TH - 1:TH],
                                        m_hi[:, 0:1], None, op0=ALU.mult)
            if pref + "_inT" in dbg:
                for kt in range(KT):
                    nc.sync.dma_start(out=dbg[pref + "_inT"][kt * 128:(kt + 1) * 128, :],
                                      in_=in_t[:, kt, :].bitcast(f32))
            c3 = actp.tile([128, KT, T], f32r, tag="c3")
            for mt in range(KT):
                wts = [stream_w(cwpool, cT[k], D, mt, tag="c_w") for k in range(3)]
                p = ps.tile([128, 512], f32, tag="pA")
                for kt in range(KT):
                    for k in range(3):
                        nc.tensor.matmul(p[:], wts[k][:, kt, :], in_t[:, kt, k:k + T],
                                         start=(kt == 0 and k == 0), stop=(kt == KT - 1 and k == 2))
                nc.vector.tensor_scalar_add(c3[:, mt, :], p[:], b_c[:, mt:mt + 1])
            if pref + "3T" in dbg:
                for kt in range(KT):
                    nc.sync.dma_start(out=dbg[pref + "3T"][kt * 128:(kt + 1) * 128, :],
                                      in_=c3[:, kt, :].bitcast(f32))
            for mt in range(KT):
                wt = stream_w(wpool, pT, 2 * D, mt, tag="proj_w")
                p = ps.tile([128, 512], f32, tag="pA")
                for kt in range(KT):
                    nc.tensor.matmul(p[:], wt[:, kt, :], in_t[:, kt, 1:1 + T],
                                     start=(kt == 0), stop=False)
                for kt in range(KT):
                    nc.tensor.matmul(p[:], wt[:, KT + kt, :], c3[:, kt, :],
                                     start=False, stop=(kt == KT - 1))
                ev = evp.tile([128, T], f32r, tag="ev")
                nc.vector.tensor_scalar_add(ev[:], p[:], b_p[:, mt:mt + 1])
                nc.sync.dma_start(out=a2a1_in[mt, qk_row0:qk_row0 + 128, :], in_=ev[:])
                if pref + "T_" in dbg:
                    nc.sync.dma_start(out=dbg[pref + "T_"][mt * 128:(mt + 1) * 128, :],
                                      in_=ev[:].bitcast(f32))

        # ---- K branch, V, tau/delta, Q branch ----
        branch(g("WkT"), b_wk, g("ckT"), b_ck, g("kpT"), b_kp, 0, "k")

        # V: natural layout (token, dim)
        for nchunk in range(2):
            wt = stream_w(vwpool, g("WvT"), D, nchunk, mwidth=512, tag="v_w")
            for tt in range(T // 128):
                p = ps.tile([128, 512], f32, tag="pA")
                for kt in range(KT):
                    nc.tensor.matmul(p[:], xr[:, kt, 1 + tt * 128:1 + tt * 128 + 128],
                                     wt[:, kt, :], start=(kt == 0), stop=(kt == KT - 1))
                ev = evp.tile([128, 512], f32r, tag="ev")
                nc.vector.tensor_tensor(ev[:], p[:], bv[:, nchunk * 512:nchunk * 512 + 512], ALU.add)
                for j in range(4):
                    hp = nchunk * 4 + j
                    vsec = a2a1_in[hp, 256:384, :].rearrange("r t -> (r t)").rearrange(
                        "(t d) -> t d", d=128)
                    nc.sync.dma_start(out=vsec[tt * 128:(tt + 1) * 128, :],
                                      in_=ev[:, j * 128:(j + 1) * 128])
                if "V_" in dbg:
                    nc.sync.dma_start(
                        out=dbg["V_"][tt * 128:(tt + 1) * 128, nchunk * 512:(nchunk + 1) * 512],
                        in_=ev[:].bitcast(f32))

        def td_path(p1name, t2name, bias_t, out_row, scale, dbg_name):
            pacc = ps_td.tile([16, 512], f32, tag="ptd")
            p1 = constp.tile([128, 16, 4], f32, tag="p1_" + p1name)
            nc.sync.dma_start(out=p1[:], in_=g(p1name).rearrange("(g p) c -> p g c", p=128))
            for gi in range(16):
                xd = tdp.tile([128, TH], f32, tag="xd")
                nc.sync.dma_start(out=xd[0:64, :], in_=xT[gi * 64:(gi + 1) * 64, :])
                nc.sync.dma_start(out=xd[64:128, :], in_=xT[gi * 64:(gi + 1) * 64, :])
                mid = tdp.tile([128, T], f32, tag="mid")
                nc.vector.tensor_scalar(mid[:], xd[:, 0:T], p1[:, gi, 0:1], None, op0=ALU.mult)
                nc.vector.scalar_tensor_tensor(mid[:], xd[:, 1:1 + T], p1[:, gi, 1:2], mid[:],
                                               op0=ALU.mult, op1=ALU.add)
                nc.vector.scalar_tensor_tensor(mid[:], xd[:, 2:2 + T], p1[:, gi, 2:3], mid[:],
                                               op0=ALU.mult, op1=ALU.add)
                gact = tdp.tile([128, T], f32r, tag="gact")
                nc.scalar.activation(out=gact[:], in_=mid[:], func=GELU_FUNC,
                                     bias=p1[:, gi, 3:4], scale=1.0)
                w2 = wpool.tile([128, 16], f32r, tag="td2")
                nc.sync.dma_start(out=w2[:], in_=g(t2name)[gi * 128:(gi + 1) * 128, :].bitcast(f32r))
                nc.tensor.matmul(pacc[:], w2[:], gact[:], start=(gi == 0), stop=(gi == 15))
            row = tdp.tile([16, T], f32, tag="td_row")
            nc.scalar.activation(out=row[:], in_=pacc[:], func=AF.Sigmoid, bias=bias_t[:, 0:1])
            rowr = tdp.tile([16, T], f32r, tag="td_rowr")
            nc.vector.tensor_scalar(rowr[:], row[:], float(scale), None, op0=ALU.mult)
            nc.sync.dma_start(out=a2a1_in[:, out_row:out_row + 2, :], in_=rowr[:])
            if dbg_name in dbg:
                nc.sync.dma_start(out=dbg[dbg_name][:], in_=rowr[:].bitcast(f32))

        td_path("tau1p", "tau2T", b_tau2, 384, 0.125, "tau")
        td_path("del1p", "del2T", b_del2, 386, 1.0, "delta")

        branch(g("WqT"), b_wq, g("cqT"), b_cq, g("qpT"), b_qp, 128, "q")

    # ---- A2A #1 ----
    nc.gpsimd.collective_compute("AllToAll", ALU.bypass, replica_groups=GROUPS,
                                 ins=[a2a1_in[:]], outs=[a2a1_out[:]])
    if "a2a1_out" in dbg:
        nc.sync.dma_start(out=dbg["a2a1_out"][:], in_=a2a1_out[:].bitcast(f32))

    # ---- Phase B: attention per (batch, head-within-pair) ----
    with (
        tc.tile_pool(name="hconst", bufs=1) as hcp,
        tc.tile_pool(name="hp", bufs=2) as hp_pool,
        tc.tile_pool(name="ep", bufs=4) as ep,
        tc.tile_pool(name="op", bufs=3) as op_pool,
        tc.tile_pool(name="ps_s", bufs=3, space="PSUM") as ps_s,
        tc.tile_pool(name="ps_o", bufs=2, space="PSUM") as ps_o,
        tc.tile_pool(name="ps_b", bufs=1, space="PSUM") as ps_b,
    ):
        ones64f = hcp.tile([1, 64], f32, tag="ones64f")
        nc.vector.memset(ones64f[:], 1.0)
        ones64 = hcp.tile([1, 64], f32r, tag="ones64")
        nc.vector.tensor_copy(out=ones64[:], in_=ones64f[:])
        onescol = hcp.tile([128, 16], f32, tag="onescol")
        nc.vector.memset(onescol[:], 1.0)
        for b in range(2):
            for hh in range(2):
                blk0 = 4 * b
                kts = hp_pool.tile([64, 4, T], f32r, tag="kts")
                nc.sync.dma_start(out=kts[:], in_=a2a1_out[blk0:blk0 + 4, hh * 64:hh * 64 + 64, :]
                                  .transpose([1, 0, 2]))
                qts = hp_pool.tile([64, 4, T], f32r, tag="qts")
                nc.sync.dma_start(out=qts[:],
                                  in_=a2a1_out[blk0:blk0 + 4, 128 + hh * 64:128 + hh * 64 + 64, :]
                                  .transpose([1, 0, 2]))
                vt = hp_pool.tile([128, 16, 65], f32r, tag="vt")
                nc.vector.tensor_copy(out=vt[:, :, 64:65], in_=onescol.unsqueeze(2))
                for j in range(4):
                    vsec = a2a1_out[blk0 + j, 256:384, :].rearrange("r t -> (r t)").rearrange(
                        "(a p d) -> p a d", p=128, d=128)
                    nc.sync.dma_start(out=vt[:, j * 4:(j + 1) * 4, 0:64],
                                      in_=vsec[:, :, hh * 64:hh * 64 + 64])
                taur = hp_pool.tile([1, 4, T], f32r, tag="taur")
                nc.sync.dma_start(out=taur[:],
                                  in_=a2a1_out[blk0:blk0 + 4, 384 + hh:384 + hh + 1, :]
                                  .transpose([1, 0, 2]))
                delt = hp_pool.tile([128, 4, 4], f32, tag="delt")
                for j in range(4):
                    nc.sync.dma_start(
                        out=delt[:, j, :],
                        in_=a2a1_out[blk0 + j, 386 + hh, :].bitcast(f32)
                        .rearrange("(a p) -> p a", p=128))
                qs = hp_pool.tile([64, 4, T], f32r, tag="qs")
                for qc in range(4):
                    pb = ps_b.tile([64, T], f32, tag="pb")
                    nc.tensor.matmul(pb[:], ones64[:], taur[:, qc, :], start=True, stop=True)
                    nc.vector.tensor_tensor(qs[:, qc, :], qts[:, qc, :], pb[:], ALU.mult)
                kflat = kts.rearrange("p a t -> p (a t)")
                for qc in range(4):
                    po = ps_o.tile([65, T], f32, tag="po")
                    for kt in range(16):
                        s = ps_s.tile([128, T], f32, tag="s")
                        nc.tensor.matmul(s[:], kflat[:, kt * 128:(kt + 1) * 128],
                                         qs[:, qc, :], start=True, stop=True)
                        e = ep.tile([128, T], f32r, tag="e")
                        nc.scalar.activation(out=e[:], in_=s[:], func=AF.Exp,
                                             bias=delt[:, kt // 4, kt % 4:kt % 4 + 1], scale=1.0)
                        nc.tensor.matmul(po[:], vt[:, kt, :], e[:],
                                         start=(kt == 0), stop=(kt == 15))
                    rs = op_pool.tile([1, T], f32r, tag="rs")
                    with nc.allow_low_precision(reason="f32r reciprocal for softmax denom"):
                        nc.vector.reciprocal(out=rs[:], in_=po[64:65, :])
                    pb2 = ps_b.tile([64, T], f32, tag="pb2")
                    nc.tensor.matmul(pb2[:], ones64[:], rs[:], start=True, stop=True)
                    rb = op_pool.tile([64, T], f32, tag="rb")
                    nc.vector.tensor_copy(out=rb[:], in_=pb2[:])
                    ot = op_pool.tile([64, T], f32r, tag="ot")
                    nc.vector.tensor_tensor(ot[:], po[0:64, :], rb[:], ALU.mult)
                    nc.sync.dma_start(out=a2a2_in[b * 4 + qc, hh * 64:hh * 64 + 64, :], in_=ot[:])

    # ---- A2A #2 ----
    nc.gpsimd.collective_compute("AllToAll", ALU.bypass, replica_groups=GROUPS,
                                 ins=[a2a2_in[:]], outs=[a2a2_out[:]])
    if "attnT" in dbg:
        nc.sync.dma_start(out=dbg["attnT"][:],
                          in_=a2a2_out.rearrange("s r t -> (s r) t").bitcast(f32))

    # ---- Phase C: out_proj ----
    with (
        tc.tile_pool(name="cw", bufs=3) as cw,
        tc.tile_pool(name="cin", bufs=1) as cin,
        tc.tile_pool(name="cev", bufs=3) as cev,
        tc.tile_pool(name="ps_c", bufs=4, space="PSUM") as ps_c,
    ):
        at = cin.tile([128, KT, T], f32r, tag="at")
        nc.sync.dma_start(out=at[:], in_=a2a2_out.rearrange("s (q p) t -> p (s q) t", p=128))
        bias_out = cin.tile([128, KT], f32, tag="bias_out2")
        nc.sync.dma_start(out=bias_out[:], in_=g("out_b").rearrange("(mt p) -> p mt", p=128))
        for mt in range(KT):
            wt = cw.tile([128, KT, 128], f32r, tag="ow")
            nc.sync.dma_start(out=wt[:], in_=g("outT")[:, mt * 128:(mt + 1) * 128]
                              .rearrange("(kt p) m -> p kt m", p=128).bitcast(f32r))
            p = ps_c.tile([128, T], f32, tag="pc")
            for kt in range(KT):
                nc.tensor.matmul(p[:], wt[:, kt, :], at[:, kt, :],
                                 start=(kt == 0), stop=(kt == KT - 1))
            ev = cev.tile([128, T], f32, tag="cev")
            nc.vector.tensor_scalar_add(ev[:], p[:], bias_out[:, mt:mt + 1])
            nc.sync.dma_start(out=yT[mt * 128:(mt + 1) * 128, :], in_=ev[:])


def make_inputs(full):
    """full: dict of original reference inputs -> list of 8 per-core in_maps."""
    x = np.asarray(full["x"], dtype=np.float32)
    common = {
        "WqT": full["Wq_w"].T, "Wq_b": full["Wq_b"],
        "WkT": full["Wk_w"].T, "Wk_b": full["Wk_b"],
        "WvT": full["Wv_w"].T, "Wv_b": full["Wv_b"],
        "cqT": np.asarray(full["convq_w"]).transpose(2, 1, 0), "cq_b": full["convq_b"],
        "ckT": np.asarray(full["convk_w"]).transpose(2, 1, 0), "ck_b": full["convk_b"],
        "qpT": full["qproj_w"].T, "qp_b": full["qproj_b"],
        "kpT": full["kproj_w"].T, "kp_b": full["kproj_b"],
        "tau1p": np.concatenate([np.asarray(full["tau1_w"])[:, 0, :],
                                 np.asarray(full["tau1_b"])[:, None]], axis=1),
        "del1p": np.concatenate([np.asarray(full["del1_w"])[:, 0, :],
                                 np.asarray(full["del1_b"])[:, None]], axis=1),
        "tau2T": np.asarray(full["tau2_w"])[:, :, 0].T, "tau2_b": full["tau2_b"],
        "del2T": np.asarray(full["del2_w"])[:, :, 0].T, "del2_b": full["del2_b"],
        "outT": full["out_w"].T, "out_b": full["out_b"],
    }
    perm = np.concatenate([g * 128 + np.concatenate([np.arange(0, 128, 2), np.arange(1, 128, 2)])
                           for g in range(16)])
    for k in ["tau1p", "del1p", "tau2T", "del2T"]:
        common[k] = np.asarray(common[k])[perm]
    common = {k: np.ascontiguousarray(np.asarray(v, dtype=np.float32)) for k, v in common.items()}
    ins = []
    for c in range(NCORES):
        b, t0 = c // 4, (c % 4) * T
        xb = np.zeros((TH, D), np.float32)
        lo, hi = max(t0 - 1, 0), min(t0 + T + 1, L)
        xb[lo - (t0 - 1):hi - (t0 - 1)] = x[b, lo:hi]
        m = dict(common)
        m["xT"] = np.ascontiguousarray(xb.T)
        m["mask_lo"] = np.array([0.0 if t0 == 0 else 1.0], np.float32)
        m["mask_hi"] = np.array([0.0 if t0 + T == L else 1.0], np.float32)
        ins.append(m)
    return ins


def assemble(results):
    y = np.empty((B, L, D), np.float32)
    for c in range(NCORES):
        b, t0 = c // 4, (c % 4) * T
        y[b, t0:t0 + T] = results[c]["yT"].T
    return y


def kernel(**inputs):
    """Takes the full unsharded reference inputs, returns the full (B, L, D) output."""
    from concourse.bass_utils import run_bass_kernel_spmd
    nc, _ = build()
    in_maps = make_inputs(inputs)
    res = run_bass_kernel_spmd(nc, in_maps, list(range(NCORES)))
    return assemble(res.results)



# revision 2
# speedup vs baseline: 1.2627x; 1.2627x over previous
"""nn_DCAttention Trainium2 kernel v2: bf16 everywhere, split+overlapped
collectives, head-parallel out_proj with ReduceScatter.

Sharding:
  Phase A (projections): token-parallel, 8 blocks of 512 tokens, all matmul
  operands bf16 (weights host-pre-tiled for fully contiguous DMA).
  cc_kv (AllToAll: K^T, V, exp(delta)) issues before the Q branch so it
  overlaps with Q compute; cc_q (Q^T pre-scaled by tau'/8) follows.
  Phase B: per (batch, head) flash attention in (dims, tokens) layout;
  exp(delta) is folded into the V/ones columns so the softmax exp is
  bias-free and batched across 2 PSUM banks per activation.
  A2A #2 (bf16) re-shards attention output back to token-parallel, then
  out_proj runs token-parallel (ReduceScatter is rejected by this runtime's
  NEFF loader, so the baseline A2A structure is kept, at half the bytes).
Output: yT bf16 (1024, 512) per core = (dims, this core's tokens);
  assemble() transposes host-side.
"""
import numpy as np
import concourse.bass as bass
import concourse.tile as tile
import concourse.mybir as mybir
from concourse import bacc

f32 = mybir.dt.float32
f32r = mybir.dt.float32r
bf16 = mybir.dt.bfloat16
AF = mybir.ActivationFunctionType
ALU = mybir.AluOpType

D, H, B, L = 1024, 16, 2, 2048
DK = D // H
NCORES = 8
T = (B * L) // NCORES      # 512
TH = T + 2
KT = D // 128              # 8
GROUPS = [[0, 1, 2, 3, 4, 5, 6, 7]]
KV_ROWS = 258              # 128 K^T + 128 V + 2 expdelta
GELU_FUNC = AF.Gelu


def build(debug_outputs=(), repeat=1):
    nc = bacc.Bacc(None, target_bir_lowering=False, debug=False)
    nc.num_devices = NCORES

    dp = lambda name, shape, dtype: nc.declare_dram_parameter(name, list(shape), dtype, isOutput=False)
    xr_d = dp("xr", (128, KT, TH), bf16)
    wq_d = dp("wq", (KT, 128, KT, 128), bf16)
    wk_d = dp("wk", (KT, 128, KT, 128), bf16)
    wv_d = dp("wv", (2, 128, KT, 512), bf16)
    cq_d = dp("cq", (KT, 128, 24, 128), bf16)
    ck_d = dp("ck", (KT, 128, 24, 128), bf16)
    qp_d = dp("qp", (KT, 128, 16, 128), bf16)
    kp_d = dp("kp", (KT, 128, 16, 128), bf16)
    ow_d = dp("ow", (KT, 128, KT, 128), bf16)
    bcol_d = dp("bcol", (128, KT, 7), f32)     # wq_b wk_b cq_b ck_b qp_b kp_b out_b
    bbrd_d = dp("bbrd", (1, D), bf16)          # wv_b
    tau1p_d = dp("tau1p", (128, 16, 4), f32)
    del1p_d = dp("del1p", (128, 16, 4), f32)
    t2w_d = dp("t2w", (128, 16, 16), bf16)
    d2w_d = dp("d2w", (128, 16, 16), bf16)
    t2b_d = dp("t2b", (16, 1), f32)
    d2b_d = dp("d2b", (16, 1), f32)
    mlo_d = dp("mlo", (1,), f32)
    mhi_d = dp("mhi", (1,), f32)

    yT = nc.declare_dram_parameter("yT", [D, T], bf16, isOutput=True)

    dbg = {}
    for name, shape in [
        ("k_inT", (D, TH)), ("k3T", (D, T)), ("kT_", (D, T)),
        ("q_inT", (D, TH)), ("q3T", (D, T)), ("qT_", (D, T)),
        ("V_", (T, D)), ("taus", (H, T)), ("expd", (H, T)),
        ("kv_out", (NCORES, KV_ROWS, T)), ("q_out", (NCORES, 128, T)),
        ("attnT", (D, T)),
    ]:
        if name in debug_outputs:
            dbg[name] = nc.declare_dram_parameter("dbg_" + name, list(shape), f32, isOutput=True)

    cc_kv_in = nc.dram_tensor("cc_kv_in", [NCORES, KV_ROWS, T], bf16)
    cc_kv_out = nc.dram_tensor("cc_kv_out", [NCORES, KV_ROWS, T], bf16)
    cc_q_in = nc.dram_tensor("cc_q_in", [NCORES, 128, T], bf16)
    cc_q_out = nc.dram_tensor("cc_q_out", [NCORES, 128, T], bf16)
    a2a2_in = nc.dram_tensor("a2a2_in", [NCORES, 128, T], bf16)
    a2a2_out = nc.dram_tensor("a2a2_out", [NCORES, 128, T], bf16)
    taud = nc.dram_tensor("taud", [H, T], bf16)

    env = dict(locals())
    with tile.TileContext(nc) as tc:
        for _rep in range(repeat):
            _body(nc, tc, env)
    nc.finalize()
    return nc, dbg


def _dbg_copy(nc, pool, dst_ap, src_ap, shape):
    """Copy a bf16 SBUF tile to an f32 debug DRAM tensor."""
    t = pool.tile(list(shape), f32, tag="dbgcp")
    nc.vector.tensor_copy(out=t[:], in_=src_ap)
    nc.sync.dma_start(out=dst_ap, in_=t[:])


def _body(nc, tc, env):
    g = lambda n: env[n]
    dbg = g("dbg")
    cc_kv_in, cc_kv_out = g("cc_kv_in"), g("cc_kv_out")
    cc_q_in, cc_q_out = g("cc_q_in"), g("cc_q_out")
    a2a2_in, a2a2_out = g("a2a2_in"), g("a2a2_out")
    yT, taud = g("yT"), g("taud")

    with (
        tc.tile_pool(name="xp", bufs=1) as xp,
        tc.tile_pool(name="const", bufs=1) as constp,
        tc.tile_pool(name="wpool", bufs=2) as wpool,      # lin weight slices
        tc.tile_pool(name="cwpool", bufs=2) as cwpool,    # conv slices
        tc.tile_pool(name="ppool", bufs=2) as ppool,      # proj slices
        tc.tile_pool(name="vwpool", bufs=2) as vwpool,
        tc.tile_pool(name="actp", bufs=1) as actp,        # in_t / c3 (reused q/k)
        tc.tile_pool(name="evp", bufs=4) as evp,
        tc.tile_pool(name="taup", bufs=2) as taup,
        tc.tile_pool(name="tdp", bufs=3) as tdp,
        tc.tile_pool(name="dbgp", bufs=2) as dbgp,
        tc.tile_pool(name="ps", bufs=6, space="PSUM") as ps,
        tc.tile_pool(name="ps_td", bufs=2, space="PSUM") as ps_td,
    ):
        # ---- x^T (bf16, host-pretiled) ----
        xr = xp.tile([128, KT, TH], bf16, tag="xr")
        nc.sync.dma_start(out=xr[:], in_=g("xr_d")[:])

        # ---- constants ----
        bcol = constp.tile([128, KT, 7], f32, tag="bcol")
        nc.sync.dma_start(out=bcol[:], in_=g("bcol_d")[:])
        bvb = constp.tile([128, D], bf16, tag="bvb")
        nc.sync.dma_start(out=bvb[:], in_=g("bbrd_d")[0].unsqueeze(0).broadcast_to([128, D]))
        t2wt = constp.tile([128, 16, 16], bf16, tag="t2wt")
        nc.sync.dma_start(out=t2wt[:], in_=g("t2w_d")[:])
        d2wt = constp.tile([128, 16, 16], bf16, tag="d2wt")
        nc.sync.dma_start(out=d2wt[:], in_=g("d2w_d")[:])
        b_tau2 = constp.tile([16, 1], f32, tag="b_tau2")
        nc.sync.dma_start(out=b_tau2[:], in_=g("t2b_d")[:])
        b_del2 = constp.tile([16, 1], f32, tag="b_del2")
        nc.sync.dma_start(out=b_del2[:], in_=g("d2b_d")[:])
        p1t = constp.tile([128, 16, 4], f32, tag="p1t")
        nc.sync.dma_start(out=p1t[:], in_=g("tau1p_d")[:])
        p1d = constp.tile([128, 16, 4], f32, tag="p1d")
        nc.sync.dma_start(out=p1d[:], in_=g("del1p_d")[:])
        m_lo = constp.tile([128, 1], f32, tag="m_lo")
        nc.sync.dma_start(out=m_lo[:], in_=g("mlo_d").ap().unsqueeze(0).broadcast_to([128, 1]))
        m_hi = constp.tile([128, 1], f32, tag="m_hi")
        nc.sync.dma_start(out=m_hi[:], in_=g("mhi_d").ap().unsqueeze(0).broadcast_to([128, 1]))

        NCH = [(0, 512), (512, 2)]

        def branch(w_d, bj, c_d, cj, p_d, pj, pref, q_tau=None):
            """linear -> conv3 -> proj. Writes proj^T to cc target rows."""
            in_t = actp.tile([128, KT, TH], bf16, tag="lin")
            for mt in range(KT):
                wt = wpool.tile([128, KT, 128], bf16, tag="lin_w")
                nc.sync.dma_start(out=wt[:], in_=w_d[mt])
                for (n0, nw) in NCH:
                    p = ps.tile([128, 512], f32, tag="pA")
                    for kt in range(KT):
                        nc.tensor.matmul(p[:, :nw], wt[:, kt, :], xr[:, kt, n0:n0 + nw],
                                         start=(kt == 0), stop=(kt == KT - 1))
                    nc.vector.tensor_scalar_add(in_t[:, mt, n0:n0 + nw], p[:, :nw],
                                                bcol[:, mt, bj:bj + 1])
                nc.vector.tensor_scalar(in_t[:, mt, 0:1], in_t[:, mt, 0:1],
                                        m_lo[:, 0:1], None, op0=ALU.mult)
                nc.vector.tensor_scalar(in_t[:, mt, TH - 1:TH], in_t[:, mt, TH - 1:TH],
                                        m_hi[:, 0:1], None, op0=ALU.mult)
            if pref + "_inT" in dbg:
                for kt in range(KT):
                    _dbg_copy(nc, dbgp, dbg[pref + "_inT"][kt * 128:(kt + 1) * 128, :],
                              in_t[:, kt, :], (128, TH))
            c3 = actp.tile([128, KT, T], bf16, tag="c3")
            for mt in range(KT):
                cwt = cwpool.tile([128, 24, 128], bf16, tag="c_w")
                nc.sync.dma_start(out=cwt[:], in_=c_d[mt])
                p = ps.tile([128, 512], f32, tag="pA")
                for j in range(24):
                    tap, kt = j // KT, j % KT
                    nc.tensor.matmul(p[:], cwt[:, j, :], in_t[:, kt, tap:tap + T],
                                     start=(j == 0), stop=(j == 23))
                nc.vector.tensor_scalar_add(c3[:, mt, :], p[:], bcol[:, mt, cj:cj + 1])
            if pref + "3T" in dbg:
                for kt in range(KT):
                    _dbg_copy(nc, dbgp, dbg[pref + "3T"][kt * 128:(kt + 1) * 128, :],
                              c3[:, kt, :], (128, T))
            for mt in range(KT):
                pwt = ppool.tile([128, 16, 128], bf16, tag="proj_w")
                nc.sync.dma_start(out=pwt[:], in_=p_d[mt])
                p = ps.tile([128, 512], f32, tag="pA")
                for kt in range(KT):
                    nc.tensor.matmul(p[:], pwt[:, kt, :], in_t[:, kt, 1:1 + T],
                                     start=(kt == 0), stop=False)
                for kt in range(KT):
                    nc.tensor.matmul(p[:], pwt[:, KT + kt, :], c3[:, kt, :],
                                     start=False, stop=(kt == KT - 1))
                ev = evp.tile([128, T], bf16, tag="ev")
                if q_tau is None:
                    nc.vector.tensor_scalar_add(ev[:], p[:], bcol[:, mt, pj:pj + 1])
                    nc.sync.dma_start(out=cc_kv_in[mt, 0:128, :], in_=ev[:])
                else:
                    tb = taup.tile([128, T], bf16, tag="taub")
                    nc.sync.dma_start(out=tb[0:64, :],
                                      in_=taud[2 * mt].unsqueeze(0).broadcast_to([64, T]))
                    nc.sync.dma_start(out=tb[64:128, :],
                                      in_=taud[2 * mt + 1].unsqueeze(0).broadcast_to([64, T]))
                    nc.vector.scalar_tensor_tensor(ev[:], p[:], bcol[:, mt, pj:pj + 1],
                                                   tb[:], op0=ALU.add, op1=ALU.mult)
                    nc.sync.dma_start(out=cc_q_in[mt, :, :], in_=ev[:])
                if pref + "T_" in dbg:
                    _dbg_copy(nc, dbgp, dbg[pref + "T_"][mt * 128:(mt + 1) * 128, :],
                              ev[:], (128, T))

        def td_path(p1, w2t, bias_t, post, dbg_name):
            """tau/delta depthwise conv -> gelu -> pointwise -> sigmoid [-> post].
            Returns [16, T] bf16 tile."""
            pacc = ps_td.tile([16, 512], f32, tag="ptd")
            for gi in range(16):
                xd = tdp.tile([128, TH], bf16, tag="xd")
                src = xr[(gi % 2) * 64:(gi % 2) * 64 + 64, gi // 2, :]
                nc.sync.dma_start(out=xd[0:64, :], in_=src)
                nc.sync.dma_start(out=xd[64:128, :], in_=src)
                mid = tdp.tile([128, T], bf16, tag="mid")
                nc.vector.tensor_scalar(mid[:], xd[:, 0:T], p1[:, gi, 0:1], None, op0=ALU.mult)
                nc.vector.scalar_tensor_tensor(mid[:], xd[:, 1:1 + T], p1[:, gi, 1:2], mid[:],
                                               op0=ALU.mult, op1=ALU.add)
                nc.vector.scalar_tensor_tensor(mid[:], xd[:, 2:2 + T], p1[:, gi, 2:3], mid[:],
                                               op0=ALU.mult, op1=ALU.add)
                gact = tdp.tile([128, T], bf16, tag="gact")
                nc.scalar.activation(out=gact[:], in_=mid[:], func=GELU_FUNC,
                                     bias=p1[:, gi, 3:4], scale=1.0)
                nc.tensor.matmul(pacc[:], w2t[:, gi, :], gact[:],
                                 start=(gi == 0), stop=(gi == 15))
            row = tdp.tile([16, T], f32, tag="td_row")
            nc.scalar.activation(out=row[:], in_=pacc[:], func=AF.Sigmoid, bias=bias_t[:, 0:1])
            out = tdp.tile([16, T], bf16, tag="td_" + dbg_name)
            post(row, out)
            if dbg_name in dbg:
                _dbg_copy(nc, dbgp, dbg[dbg_name][:], out[:], (16, T))
            return out

        # ---- K branch, V, delta -> cc_kv; tau; Q branch -> cc_q ----
        branch(g("wk_d"), 1, g("ck_d"), 3, g("kp_d"), 5, "k")

        for nchunk in range(2):
            vwt = vwpool.tile([128, KT, 512], bf16, tag="v_w")
            nc.sync.dma_start(out=vwt[:], in_=g("wv_d")[nchunk])
            for tt in range(4):
                p = ps.tile([128, 512], f32, tag="pA")
                for kt in range(KT):
                    nc.tensor.matmul(p[:], xr[:, kt, 1 + tt * 128:1 + tt * 128 + 128],
                                     vwt[:, kt, :], start=(kt == 0), stop=(kt == KT - 1))
                ev = evp.tile([128, 512], bf16, tag="ev")
                nc.vector.tensor_tensor(ev[:], p[:], bvb[:, nchunk * 512:nchunk * 512 + 512],
                                        ALU.add)
                for j in range(4):
                    hp = nchunk * 4 + j
                    vsec = cc_kv_in[hp, 128:256, :].rearrange("r t -> (r t)").rearrange(
                        "(t d) -> t d", d=128)
                    nc.sync.dma_start(out=vsec[tt * 128:(tt + 1) * 128, :],
                                      in_=ev[:, j * 128:(j + 1) * 128])
                if "V_" in dbg:
                    _dbg_copy(nc, dbgp,
                              dbg["V_"][tt * 128:(tt + 1) * 128,
                                        nchunk * 512:(nchunk + 1) * 512],
                              ev[:], (128, 512))

        def post_delta(row, out):
            nc.scalar.activation(out=out[:], in_=row[:], func=AF.Exp, bias=0.0, scale=1.0)
        expd_row = td_path(p1d, d2wt, b_del2, post_delta, "expd")
        nc.sync.dma_start(out=cc_kv_in[:, 256:258, :], in_=expd_row[:])

        nc.gpsimd.collective_compute("AllToAll", ALU.bypass, replica_groups=GROUPS,
                                     ins=[cc_kv_in[:]], outs=[cc_kv_out[:]])

        def post_tau(row, out):
            nc.vector.tensor_scalar(out[:], row[:], 0.125, None, op0=ALU.mult)
        tau_row = td_path(p1t, t2wt, b_tau2, post_tau, "taus")
        nc.sync.dma_start(out=taud[:], in_=tau_row[:])

        branch(g("wq_d"), 0, g("cq_d"), 2, g("qp_d"), 4, "q", q_tau=True)

    nc.gpsimd.collective_compute("AllToAll", ALU.bypass, replica_groups=GROUPS,
                                 ins=[cc_q_in[:]], outs=[cc_q_out[:]])
    if "kv_out" in dbg:
        with tc.tile_pool(name="dk", bufs=2) as dk:
            for s in range(NCORES):
                t_ = dk.tile([KV_ROWS, T], f32, tag="dkc")
                tb_ = dk.tile([KV_ROWS, T], bf16, tag="dkb")
                nc.sync.dma_start(out=tb_[:], in_=cc_kv_out[s])
                nc.vector.tensor_copy(out=t_[:], in_=tb_[:])
                nc.sync.dma_start(out=dbg["kv_out"][s], in_=t_[:])
    if "q_out" in dbg:
        with tc.tile_pool(name="dq", bufs=2) as dq:
            for s in range(NCORES):
                t_ = dq.tile([128, T], f32, tag="dqc")
                tb_ = dq.tile([128, T], bf16, tag="dqb")
                nc.sync.dma_start(out=tb_[:], in_=cc_q_out[s])
                nc.vector.tensor_copy(out=t_[:], in_=tb_[:])
                nc.sync.dma_start(out=dbg["q_out"][s], in_=t_[:])

    # ---- Phase B ----
    with (
        tc.tile_pool(name="hconst", bufs=1) as hcp,
        tc.tile_pool(name="hp", bufs=2) as hp_pool,
        tc.tile_pool(name="ep", bufs=3) as ep,
        tc.tile_pool(name="op", bufs=3) as op_pool,
        tc.tile_pool(name="ps_s", bufs=2, space="PSUM") as ps_s,
        tc.tile_pool(name="ps_o", bufs=2, space="PSUM") as ps_o,
        tc.tile_pool(name="ps_m", bufs=2, space="PSUM") as ps_m,
    ):
        ones64f = hcp.tile([1, 64], f32, tag="ones64f")
        nc.vector.memset(ones64f[:], 1.0)
        ones64 = hcp.tile([1, 64], f32r, tag="ones64")
        nc.vector.tensor_copy(out=ones64[:], in_=ones64f[:])

        # Software pipeline state: `prev` defers the AV matmuls of score
        # group g until after the scores of group g+1 are emitted (the
        # in-order PE then never waits on ACT exp); `pend` defers softmax
        # normalization of query-chunk qc until two groups into qc+1.
        prev = None   # (po, e2, grp, vts)
        pend = None   # (po, b, hh, qc)

        def flush_av(st):
            po_, e2_, grp_, vts_ = st
            for i2 in range(2):
                kt = grp_ * 2 + i2
                nc.tensor.matmul(po_[:], vts_[:, kt, :], e2_[:, i2, :],
                                 start=(kt == 0), stop=(kt == 15))

        def normalize(st):
            po_, b_, hh_, qc_ = st
            rs = op_pool.tile([1, T], f32r, tag="rs")
            with nc.allow_low_precision(reason="f32r reciprocal for softmax denom"):
                nc.vector.reciprocal(out=rs[:], in_=po_[64:65, :])
            pb2 = ps_m.tile([64, T], f32, tag="pb2")
            nc.tensor.matmul(pb2[:], ones64[:], rs[:], start=True, stop=True)
            rb = op_pool.tile([64, T], bf16, tag="rb")
            nc.vector.tensor_copy(out=rb[:], in_=pb2[:])
            ot = op_pool.tile([64, T], bf16, tag="ot")
            nc.vector.tensor_tensor(ot[:], po_[0:64, :], rb[:], ALU.mult)
            nc.sync.dma_start(out=a2a2_in[b_ * 4 + qc_, hh_ * 64:hh_ * 64 + 64, :],
                              in_=ot[:])

        for b in range(2):
            blk0 = 4 * b
            for hh in range(2):
                kts = hp_pool.tile([64, 4, T], bf16, tag="kts")
                nc.sync.dma_start(out=kts[:],
                                  in_=cc_kv_out[blk0:blk0 + 4, hh * 64:hh * 64 + 64, :]
                                  .transpose([1, 0, 2]))
                qts = hp_pool.tile([64, 4, T], bf16, tag="qts")
                nc.sync.dma_start(out=qts[:],
                                  in_=cc_q_out[blk0:blk0 + 4, hh * 64:hh * 64 + 64, :]
                                  .transpose([1, 0, 2]))
                vt = hp_pool.tile([128, 16, 65], bf16, tag="vt")
                nc.vector.memset(vt[:, :, 64:65], 1.0)
                for j in range(4):
                    vsec = cc_kv_out[blk0 + j, 128:256, :].rearrange("r t -> (r t)").rearrange(
                        "(a p d) -> p a d", p=128, d=128)
                    nc.sync.dma_start(out=vt[:, j * 4:(j + 1) * 4, 0:64],
                                      in_=vsec[:, :, hh * 64:hh * 64 + 64])
                delt = hp_pool.tile([128, 16], bf16, tag="delt")
                for j in range(4):
                    nc.sync.dma_start(out=delt[:, j * 4:(j + 1) * 4],
                                      in_=cc_kv_out[blk0 + j, 256 + hh, :]
                                      .rearrange("(a p) -> p a", p=128))
                deltf = hp_pool.tile([128, 16], f32, tag="deltf")
                nc.vector.tensor_copy(out=deltf[:], in_=delt[:])
                vts = hp_pool.tile([128, 16, 65], bf16, tag="vts")
                for kt in range(16):
                    nc.vector.tensor_scalar(vts[:, kt, :], vt[:, kt, :],
                                            deltf[:, kt:kt + 1], None, op0=ALU.mult)
                kflat = kts.rearrange("p a t -> p (a t)")
                for qc in range(4):
                    po = ps_o.tile([65, T], f32, tag="po")
                    for grp in range(8):
                        s2 = ps_s.tile([128, 2, T], f32, tag="s2")
                        for i2 in range(2):
                            kt = grp * 2 + i2
                            nc.tensor.matmul(s2[:, i2, :], kflat[:, kt * 128:(kt + 1) * 128],
                                             qts[:, qc, :], start=True, stop=True)
                        if prev is not None:
                            flush_av(prev)
                            prev = None
                        if grp == 1 and pend is not None:
                            normalize(pend)
                            pend = None
                        e2 = ep.tile([128, 2, T], bf16, tag="e2")
                        nc.scalar.activation(out=e2[:], in_=s2[:], func=AF.Exp,
                                             bias=0.0, scale=1.0)
                        prev = (po, e2, grp, vts)
                    pend = (po, b, hh, qc)
        flush_av(prev)
        prev = None
        normalize(pend)
        pend = None

    # ---- A2A #2 ----
    nc.gpsimd.collective_compute("AllToAll", ALU.bypass, replica_groups=GROUPS,
                                 ins=[a2a2_in[:]], outs=[a2a2_out[:]])

    # ---- Phase C: token-parallel out_proj ----
    with (
        tc.tile_pool(name="cw", bufs=2) as cw,
        tc.tile_pool(name="cin", bufs=1) as cin,
        tc.tile_pool(name="cev", bufs=3) as cev,
        tc.tile_pool(name="cdbg", bufs=2) as cdbg,
        tc.tile_pool(name="ps_c", bufs=4, space="PSUM") as ps_c,
    ):
        at = cin.tile([128, KT, T], bf16, tag="at")
        nc.sync.dma_start(out=at[:], in_=a2a2_out.rearrange("s (q p) t -> p (s q) t", p=128))
        if "attnT" in dbg:
            for kt in range(KT):
                _dbg_copy(nc, cdbg, dbg["attnT"][kt * 128:(kt + 1) * 128, :],
                          at[:, kt, :], (128, T))
        bcol2 = cin.tile([128, KT, 7], f32, tag="bcol2")
        nc.sync.dma_start(out=bcol2[:], in_=g("bcol_d")[:])
        for mt in range(KT):
            wt = cw.tile([128, KT, 128], bf16, tag="ow")
            nc.sync.dma_start(out=wt[:], in_=g("ow_d")[mt])
            p = ps_c.tile([128, T], f32, tag="pc")
            for kt in range(KT):
                nc.tensor.matmul(p[:], wt[:, kt, :], at[:, kt, :],
                                 start=(kt == 0), stop=(kt == KT - 1))
            ev = cev.tile([128, T], bf16, tag="cev")
            nc.vector.tensor_scalar_add(ev[:], p[:], bcol2[:, mt, 6:7])
            nc.sync.dma_start(out=yT[mt * 128:(mt + 1) * 128, :], in_=ev[:])


def make_inputs(full):
    """full: dict of original reference inputs -> list of 8 per-core in_maps."""
    import ml_dtypes
    bf = lambda a: np.ascontiguousarray(np.asarray(a, np.float32)).astype(ml_dtypes.bfloat16)
    f = lambda a: np.ascontiguousarray(np.asarray(a, dtype=np.float32))
    x = np.asarray(full["x"], dtype=np.float32)

    def tile_w(WT, nk):  # WT (nk*128, D) -> (KT, 128, nk, 128)
        return np.ascontiguousarray(
            WT.reshape(nk, 128, KT, 128).transpose(2, 1, 0, 3))

    def lin_w(w):        # torch (out, in) -> tiled W.T
        return tile_w(np.asarray(w, np.float32).T, KT)

    def conv_w(w):       # (D out, D in, 3) -> (KT mt, 128 p, 24 (tap,kt), 128 m)
        wt = np.asarray(w, np.float32).transpose(2, 1, 0)      # (tap, in, out)
        wt = wt.reshape(3, KT, 128, KT, 128)                   # tap, kt, p, mt, m
        return np.ascontiguousarray(wt.transpose(3, 2, 0, 1, 4).reshape(KT, 128, 24, 128))

    def proj_w(w):       # (D out, 2D in) -> (KT, 128, 16, 128)
        return tile_w(np.asarray(w, np.float32).T, 16)

    wv = np.asarray(full["Wv_w"], np.float32).T                # (in, out)
    wv_t = np.ascontiguousarray(
        wv.reshape(KT, 128, 2, 512).transpose(2, 1, 0, 3))     # (2, 128, KT, 512)

    bcol = np.stack([f(full[k]).reshape(KT, 128).T for k in
                     ["Wq_b", "Wk_b", "convq_b", "convk_b", "qproj_b", "kproj_b",
                      "out_b"]], axis=2)                        # (128, KT, 7)
    bbrd = f(full["Wv_b"]).reshape(1, D)

    perm = np.concatenate([gr * 128 + np.concatenate([np.arange(0, 128, 2),
                                                      np.arange(1, 128, 2)])
                           for gr in range(16)])
    tau1p = np.concatenate([np.asarray(full["tau1_w"], np.float32)[:, 0, :],
                            np.asarray(full["tau1_b"], np.float32)[:, None]], axis=1)[perm]
    del1p = np.concatenate([np.asarray(full["del1_w"], np.float32)[:, 0, :],
                            np.asarray(full["del1_b"], np.float32)[:, None]], axis=1)[perm]
    t2w = np.asarray(full["tau2_w"], np.float32)[:, :, 0].T[perm]  # (2048, 16)
    d2w = np.asarray(full["del2_w"], np.float32)[:, :, 0].T[perm]
    arr3 = lambda a: np.ascontiguousarray(a.reshape(16, 128, a.shape[-1]).transpose(1, 0, 2))

    common = {
        "wq": lin_w(full["Wq_w"]), "wk": lin_w(full["Wk_w"]), "wv": wv_t,
        "cq": conv_w(full["convq_w"]), "ck": conv_w(full["convk_w"]),
        "qp": proj_w(full["qproj_w"]), "kp": proj_w(full["kproj_w"]),
        "ow": lin_w(full["out_w"]),
        "bcol": bcol, "bbrd": bbrd,
        "tau1p": arr3(tau1p), "del1p": arr3(del1p),
        "t2w": arr3(t2w), "d2w": arr3(d2w),
        "t2b": f(full["tau2_b"]).reshape(16, 1), "d2b": f(full["del2_b"]).reshape(16, 1),
    }
    for k in ["wq", "wk", "wv", "cq", "ck", "qp", "kp", "ow", "bbrd", "t2w", "d2w"]:
        common[k] = bf(common[k])

    ins = []
    for c in range(NCORES):
        b, t0 = c // 4, (c % 4) * T
        xb = np.zeros((TH, D), np.float32)
        lo, hi = max(t0 - 1, 0), min(t0 + T + 1, L)
        xb[lo - (t0 - 1):hi - (t0 - 1)] = x[b, lo:hi]
        xrt = np.ascontiguousarray(xb.T.reshape(KT, 128, TH).transpose(1, 0, 2))
        m = dict(common)
        m["xr"] = bf(xrt)
        m["mlo"] = np.array([0.0 if t0 == 0 else 1.0], np.float32)
        m["mhi"] = np.array([0.0 if t0 + T == L else 1.0], np.float32)
        ins.append(m)
    return ins


def assemble(results):
    y = np.empty((B, L, D), np.float32)
    for c in range(NCORES):
        b, t0 = c // 4, (c % 4) * T
        y[b, t0:t0 + T] = np.asarray(results[c]["yT"], dtype=np.float32).T
    return y


def kernel(**inputs):
    """Takes the full unsharded reference inputs, returns the full (B, L, D) output."""
    from concourse.bass_utils import run_bass_kernel_spmd
    nc, _ = build()
    in_maps = make_inputs(inputs)
    res = run_bass_kernel_spmd(nc, in_maps, list(range(NCORES)))
    return assemble(res.results)


# revision 3
# speedup vs baseline: 1.5496x; 1.2272x over previous
"""nn_DCAttention Trainium2 kernel v2: bf16 everywhere, split+overlapped
collectives, software-pipelined flash attention.

Sharding:
  Phase A (projections): token-parallel, 8 blocks of 512 tokens, all matmul
  operands bf16 (weights host-pre-tiled for fully contiguous DMA).
  cc_kv (AllToAll: K^T, V, exp(delta)) issues before the Q branch so it
  overlaps with Q compute; cc_q (Q^T pre-scaled by tau'/8) follows.
  Phase B: per (batch, head) flash attention in (dims, tokens) layout;
  exp(delta) is folded into the V/ones columns so the softmax exp is
  bias-free and batched across 2 PSUM banks per activation.
  A2A #2 (bf16) re-shards attention output back to token-parallel, then
  out_proj runs token-parallel (ReduceScatter is rejected by this runtime's
  NEFF loader, so the baseline A2A structure is kept, at half the bytes).
Output: yT bf16 (1024, 512) per core = (dims, this core's tokens);
  assemble() transposes host-side.
"""
import numpy as np
import concourse.bass as bass
import concourse.tile as tile
import concourse.mybir as mybir
from concourse import bacc

f32 = mybir.dt.float32
f32r = mybir.dt.float32r
bf16 = mybir.dt.bfloat16
AF = mybir.ActivationFunctionType
ALU = mybir.AluOpType

D, H, B, L = 1024, 16, 2, 2048
DK = D // H
NCORES = 8
T = (B * L) // NCORES      # 512
TH = T + 2
KT = D // 128              # 8
GROUPS = [[0, 1, 2, 3, 4, 5, 6, 7]]
KV_ROWS = 258              # 128 K^T + 128 V + 2 expdelta
GELU_FUNC = AF.Gelu


def build(debug_outputs=(), repeat=1):
    nc = bacc.Bacc(None, target_bir_lowering=False, debug=False)
    nc.num_devices = NCORES

    dp = lambda name, shape, dtype: nc.declare_dram_parameter(name, list(shape), dtype, isOutput=False)
    xr_d = dp("xr", (128, KT, TH), bf16)
    wq_d = dp("wq", (KT, 128, KT, 128), bf16)
    wk_d = dp("wk", (KT, 128, KT, 128), bf16)
    wv_d = dp("wv", (2, 128, KT, 512), bf16)
    cq_d = dp("cq", (KT, 128, 24, 128), bf16)
    ck_d = dp("ck", (KT, 128, 24, 128), bf16)
    qp_d = dp("qp", (KT, 128, 16, 128), bf16)
    kp_d = dp("kp", (KT, 128, 16, 128), bf16)
    ow_d = dp("ow", (KT, 128, KT, 128), bf16)
    bcol_d = dp("bcol", (128, KT, 7), f32)     # wq_b wk_b cq_b ck_b qp_b kp_b out_b
    bbrd_d = dp("bbrd", (1, D), bf16)          # wv_b
    tau1p_d = dp("tau1p", (128, 16, 4), f32)
    del1p_d = dp("del1p", (128, 16, 4), f32)
    t2w_d = dp("t2w", (128, 16, 16), bf16)
    d2w_d = dp("d2w", (128, 16, 16), bf16)
    t2b_d = dp("t2b", (16, 1), f32)
    d2b_d = dp("d2b", (16, 1), f32)
    mlo_d = dp("mlo", (1,), f32)
    mhi_d = dp("mhi", (1,), f32)

    yT = nc.declare_dram_parameter("yT", [D, T], bf16, isOutput=True)

    dbg = {}
    for name, shape in [
        ("k_inT", (D, TH)), ("k3T", (D, T)), ("kT_", (D, T)),
        ("q_inT", (D, TH)), ("q3T", (D, T)), ("qT_", (D, T)),
        ("V_", (T, D)), ("taus", (H, T)), ("expd", (H, T)),
        ("kv_out", (NCORES, KV_ROWS, T)), ("q_out", (NCORES, 128, T)),
        ("attnT", (D, T)),
    ]:
        if name in debug_outputs:
            dbg[name] = nc.declare_dram_parameter("dbg_" + name, list(shape), f32, isOutput=True)

    cc_kv_in = nc.dram_tensor("cc_kv_in", [NCORES, KV_ROWS, T], bf16)
    cc_kv_out = nc.dram_tensor("cc_kv_out", [NCORES, KV_ROWS, T], bf16)
    cc_q_in = nc.dram_tensor("cc_q_in", [NCORES, 128, T], bf16)
    cc_q_out = nc.dram_tensor("cc_q_out", [NCORES, 128, T], bf16)
    a2a2_in = nc.dram_tensor("a2a2_in", [NCORES, 128, T], bf16)
    a2a2_out = nc.dram_tensor("a2a2_out", [NCORES, 128, T], bf16)
    taud = nc.dram_tensor("taud", [H, T], bf16)

    env = dict(locals())
    with tile.TileContext(nc) as tc:
        for _rep in range(repeat):
            _body(nc, tc, env)
    nc.finalize()
    return nc, dbg


def _dbg_copy(nc, pool, dst_ap, src_ap, shape):
    """Copy a bf16 SBUF tile to an f32 debug DRAM tensor."""
    t = pool.tile(list(shape), f32, tag="dbgcp")
    nc.vector.tensor_copy(out=t[:], in_=src_ap)
    nc.sync.dma_start(out=dst_ap, in_=t[:])


def _body(nc, tc, env):
    g = lambda n: env[n]
    dbg = g("dbg")
    cc_kv_in, cc_kv_out = g("cc_kv_in"), g("cc_kv_out")
    cc_q_in, cc_q_out = g("cc_q_in"), g("cc_q_out")
    a2a2_in, a2a2_out = g("a2a2_in"), g("a2a2_out")
    yT, taud = g("yT"), g("taud")

    with (
        tc.tile_pool(name="xp", bufs=1) as xp,
        tc.tile_pool(name="const", bufs=1) as constp,
        tc.tile_pool(name="wpool", bufs=2) as wpool,      # lin weight slices
        tc.tile_pool(name="cwpool", bufs=2) as cwpool,    # conv slices
        tc.tile_pool(name="ppool", bufs=2) as ppool,      # proj slices
        tc.tile_pool(name="vwpool", bufs=2) as vwpool,
        tc.tile_pool(name="actp", bufs=1) as actp,        # in_t / c3 (reused q/k)
        tc.tile_pool(name="evp", bufs=4) as evp,
        tc.tile_pool(name="taup", bufs=2) as taup,
        tc.tile_pool(name="tdp", bufs=3) as tdp,
        tc.tile_pool(name="dbgp", bufs=2) as dbgp,
        tc.tile_pool(name="ps", bufs=6, space="PSUM") as ps,
        tc.tile_pool(name="ps_td", bufs=2, space="PSUM") as ps_td,
    ):
        # ---- x^T (bf16, host-pretiled) ----
        xr = xp.tile([128, KT, TH], bf16, tag="xr")
        nc.sync.dma_start(out=xr[:], in_=g("xr_d")[:])

        # ---- constants ----
        bcol = constp.tile([128, KT, 7], f32, tag="bcol")
        nc.sync.dma_start(out=bcol[:], in_=g("bcol_d")[:])
        bvb = constp.tile([128, D], bf16, tag="bvb")
        nc.sync.dma_start(out=bvb[:], in_=g("bbrd_d")[0].unsqueeze(0).broadcast_to([128, D]))
        t2wt = constp.tile([128, 16, 16], bf16, tag="t2wt")
        nc.sync.dma_start(out=t2wt[:], in_=g("t2w_d")[:])
        d2wt = constp.tile([128, 16, 16], bf16, tag="d2wt")
        nc.sync.dma_start(out=d2wt[:], in_=g("d2w_d")[:])
        b_tau2 = constp.tile([16, 1], f32, tag="b_tau2")
        nc.sync.dma_start(out=b_tau2[:], in_=g("t2b_d")[:])
        b_del2 = constp.tile([16, 1], f32, tag="b_del2")
        nc.sync.dma_start(out=b_del2[:], in_=g("d2b_d")[:])
        p1t = constp.tile([128, 16, 4], f32, tag="p1t")
        nc.sync.dma_start(out=p1t[:], in_=g("tau1p_d")[:])
        p1d = constp.tile([128, 16, 4], f32, tag="p1d")
        nc.sync.dma_start(out=p1d[:], in_=g("del1p_d")[:])
        m_lo = constp.tile([128, 1], f32, tag="m_lo")
        nc.sync.dma_start(out=m_lo[:], in_=g("mlo_d").ap().unsqueeze(0).broadcast_to([128, 1]))
        m_hi = constp.tile([128, 1], f32, tag="m_hi")
        nc.sync.dma_start(out=m_hi[:], in_=g("mhi_d").ap().unsqueeze(0).broadcast_to([128, 1]))

        NCH = [(0, 512), (512, 2)]

        def branch(w_d, bj, c_d, cj, p_d, pj, pref, q_tau=None):
            """linear -> conv3 -> proj. Writes proj^T to cc target rows."""
            in_t = actp.tile([128, KT, TH], bf16, tag="lin")
            for mt in range(KT):
                wt = wpool.tile([128, KT, 128], bf16, tag="lin_w")
                nc.sync.dma_start(out=wt[:], in_=w_d[mt])
                for (n0, nw) in NCH:
                    p = ps.tile([128, 512], f32, tag="pA")
                    for kt in range(KT):
                        nc.tensor.matmul(p[:, :nw], wt[:, kt, :], xr[:, kt, n0:n0 + nw],
                                         start=(kt == 0), stop=(kt == KT - 1))
                    nc.vector.tensor_scalar_add(in_t[:, mt, n0:n0 + nw], p[:, :nw],
                                                bcol[:, mt, bj:bj + 1])
                nc.vector.tensor_scalar(in_t[:, mt, 0:1], in_t[:, mt, 0:1],
                                        m_lo[:, 0:1], None, op0=ALU.mult)
                nc.vector.tensor_scalar(in_t[:, mt, TH - 1:TH], in_t[:, mt, TH - 1:TH],
                                        m_hi[:, 0:1], None, op0=ALU.mult)
            if pref + "_inT" in dbg:
                for kt in range(KT):
                    _dbg_copy(nc, dbgp, dbg[pref + "_inT"][kt * 128:(kt + 1) * 128, :],
                              in_t[:, kt, :], (128, TH))
            c3 = actp.tile([128, KT, T], bf16, tag="c3")
            for mt in range(KT):
                cwt = cwpool.tile([128, 24, 128], bf16, tag="c_w")
                nc.sync.dma_start(out=cwt[:], in_=c_d[mt])
                p = ps.tile([128, 512], f32, tag="pA")
                for j in range(24):
                    tap, kt = j // KT, j % KT
                    nc.tensor.matmul(p[:], cwt[:, j, :], in_t[:, kt, tap:tap + T],
                                     start=(j == 0), stop=(j == 23))
                nc.vector.tensor_scalar_add(c3[:, mt, :], p[:], bcol[:, mt, cj:cj + 1])
            if pref + "3T" in dbg:
                for kt in range(KT):
                    _dbg_copy(nc, dbgp, dbg[pref + "3T"][kt * 128:(kt + 1) * 128, :],
                              c3[:, kt, :], (128, T))
            for mt in range(KT):
                pwt = ppool.tile([128, 16, 128], bf16, tag="proj_w")
                nc.sync.dma_start(out=pwt[:], in_=p_d[mt])
                p = ps.tile([128, 512], f32, tag="pA")
                for kt in range(KT):
                    nc.tensor.matmul(p[:], pwt[:, kt, :], in_t[:, kt, 1:1 + T],
                                     start=(kt == 0), stop=False)
                for kt in range(KT):
                    nc.tensor.matmul(p[:], pwt[:, KT + kt, :], c3[:, kt, :],
                                     start=False, stop=(kt == KT - 1))
                ev = evp.tile([128, T], bf16, tag="ev")
                if q_tau is None:
                    nc.vector.tensor_scalar_add(ev[:], p[:], bcol[:, mt, pj:pj + 1])
                    nc.sync.dma_start(out=cc_kv_in[mt, 0:128, :], in_=ev[:])
                else:
                    tb = taup.tile([128, T], bf16, tag="taub")
                    nc.sync.dma_start(out=tb[0:64, :],
                                      in_=taud[2 * mt].unsqueeze(0).broadcast_to([64, T]))
                    nc.sync.dma_start(out=tb[64:128, :],
                                      in_=taud[2 * mt + 1].unsqueeze(0).broadcast_to([64, T]))
                    nc.vector.scalar_tensor_tensor(ev[:], p[:], bcol[:, mt, pj:pj + 1],
                                                   tb[:], op0=ALU.add, op1=ALU.mult)
                    nc.sync.dma_start(out=cc_q_in[mt, :, :], in_=ev[:])
                if pref + "T_" in dbg:
                    _dbg_copy(nc, dbgp, dbg[pref + "T_"][mt * 128:(mt + 1) * 128, :],
                              ev[:], (128, T))

        def td_path(p1, w2t, bias_t, post, dbg_name):
            """tau/delta depthwise conv -> gelu -> pointwise -> sigmoid [-> post].
            Returns [16, T] bf16 tile."""
            pacc = ps_td.tile([16, 512], f32, tag="ptd")
            for gi in range(16):
                xd = tdp.tile([128, TH], bf16, tag="xd")
                src = xr[(gi % 2) * 64:(gi % 2) * 64 + 64, gi // 2, :]
                nc.sync.dma_start(out=xd[0:64, :], in_=src)
                nc.sync.dma_start(out=xd[64:128, :], in_=src)
                mid = tdp.tile([128, T], bf16, tag="mid")
                nc.vector.tensor_scalar(mid[:], xd[:, 0:T], p1[:, gi, 0:1], None, op0=ALU.mult)
                nc.vector.scalar_tensor_tensor(mid[:], xd[:, 1:1 + T], p1[:, gi, 1:2], mid[:],
                                               op0=ALU.mult, op1=ALU.add)
                nc.vector.scalar_tensor_tensor(mid[:], xd[:, 2:2 + T], p1[:, gi, 2:3], mid[:],
                                               op0=ALU.mult, op1=ALU.add)
                gact = tdp.tile([128, T], bf16, tag="gact")
                nc.scalar.activation(out=gact[:], in_=mid[:], func=GELU_FUNC,
                                     bias=p1[:, gi, 3:4], scale=1.0)
                nc.tensor.matmul(pacc[:], w2t[:, gi, :], gact[:],
                                 start=(gi == 0), stop=(gi == 15))
            row = tdp.tile([16, T], f32, tag="td_row")
            nc.scalar.activation(out=row[:], in_=pacc[:], func=AF.Sigmoid, bias=bias_t[:, 0:1])
            out = tdp.tile([16, T], bf16, tag="td_" + dbg_name)
            post(row, out)
            if dbg_name in dbg:
                _dbg_copy(nc, dbgp, dbg[dbg_name][:], out[:], (16, T))
            return out

        # ---- K branch, V, delta -> cc_kv; tau; Q branch -> cc_q ----
        branch(g("wk_d"), 1, g("ck_d"), 3, g("kp_d"), 5, "k")

        for nchunk in range(2):
            vwt = vwpool.tile([128, KT, 512], bf16, tag="v_w")
            nc.sync.dma_start(out=vwt[:], in_=g("wv_d")[nchunk])
            for tt in range(4):
                p = ps.tile([128, 512], f32, tag="pA")
                for kt in range(KT):
                    nc.tensor.matmul(p[:], xr[:, kt, 1 + tt * 128:1 + tt * 128 + 128],
                                     vwt[:, kt, :], start=(kt == 0), stop=(kt == KT - 1))
                ev = evp.tile([128, 512], bf16, tag="ev")
                nc.vector.tensor_tensor(ev[:], p[:], bvb[:, nchunk * 512:nchunk * 512 + 512],
                                        ALU.add)
                for j in range(4):
                    hp = nchunk * 4 + j
                    vsec = cc_kv_in[hp, 128:256, :].rearrange("r t -> (r t)").rearrange(
                        "(t d) -> t d", d=128)
                    nc.sync.dma_start(out=vsec[tt * 128:(tt + 1) * 128, :],
                                      in_=ev[:, j * 128:(j + 1) * 128])
                if "V_" in dbg:
                    _dbg_copy(nc, dbgp,
                              dbg["V_"][tt * 128:(tt + 1) * 128,
                                        nchunk * 512:(nchunk + 1) * 512],
                              ev[:], (128, 512))

        def post_delta(row, out):
            nc.scalar.activation(out=out[:], in_=row[:], func=AF.Exp, bias=0.0, scale=1.0)
        expd_row = td_path(p1d, d2wt, b_del2, post_delta, "expd")
        nc.sync.dma_start(out=cc_kv_in[:, 256:258, :], in_=expd_row[:])

        nc.gpsimd.collective_compute("AllToAll", ALU.bypass, replica_groups=GROUPS,
                                     ins=[cc_kv_in[:]], outs=[cc_kv_out[:]])

        def post_tau(row, out):
            nc.vector.tensor_scalar(out[:], row[:], 0.125, None, op0=ALU.mult)
        tau_row = td_path(p1t, t2wt, b_tau2, post_tau, "taus")
        nc.sync.dma_start(out=taud[:], in_=tau_row[:])

        branch(g("wq_d"), 0, g("cq_d"), 2, g("qp_d"), 4, "q", q_tau=True)

    nc.gpsimd.collective_compute("AllToAll", ALU.bypass, replica_groups=GROUPS,
                                 ins=[cc_q_in[:]], outs=[cc_q_out[:]])
    if "kv_out" in dbg:
        with tc.tile_pool(name="dk", bufs=2) as dk:
            for s in range(NCORES):
                t_ = dk.tile([KV_ROWS, T], f32, tag="dkc")
                tb_ = dk.tile([KV_ROWS, T], bf16, tag="dkb")
                nc.sync.dma_start(out=tb_[:], in_=cc_kv_out[s])
                nc.vector.tensor_copy(out=t_[:], in_=tb_[:])
                nc.sync.dma_start(out=dbg["kv_out"][s], in_=t_[:])
    if "q_out" in dbg:
        with tc.tile_pool(name="dq", bufs=2) as dq:
            for s in range(NCORES):
                t_ = dq.tile([128, T], f32, tag="dqc")
                tb_ = dq.tile([128, T], bf16, tag="dqb")
                nc.sync.dma_start(out=tb_[:], in_=cc_q_out[s])
                nc.vector.tensor_copy(out=t_[:], in_=tb_[:])
                nc.sync.dma_start(out=dbg["q_out"][s], in_=t_[:])

    # ---- Phase B ----
    with (
        tc.tile_pool(name="hconst", bufs=1) as hcp,
        tc.tile_pool(name="hp", bufs=2) as hp_pool,
        tc.tile_pool(name="ep", bufs=3) as ep,
        tc.tile_pool(name="op", bufs=3) as op_pool,
        tc.tile_pool(name="ps_s", bufs=2, space="PSUM") as ps_s,
        tc.tile_pool(name="ps_o", bufs=2, space="PSUM") as ps_o,
        tc.tile_pool(name="ps_m", bufs=2, space="PSUM") as ps_m,
    ):
        ones64f = hcp.tile([1, 64], f32, tag="ones64f")
        nc.vector.memset(ones64f[:], 1.0)
        ones64 = hcp.tile([1, 64], f32r, tag="ones64")
        nc.vector.tensor_copy(out=ones64[:], in_=ones64f[:])

        # Software pipeline state: `prev` defers the AV matmuls of score
        # group g until after the scores of group g+1 are emitted (the
        # in-order PE then never waits on ACT exp); `pend` defers softmax
        # normalization of query-chunk qc until two groups into qc+1.
        prev = None   # (po, e2, grp, vts)
        pend = None   # (po, b, hh, qc)

        def flush_av(st):
            po_, e2_, grp_, vts_ = st
            for i2 in range(2):
                kt = grp_ * 2 + i2
                nc.tensor.matmul(po_[:], vts_[:, kt, :], e2_[:, i2, :],
                                 start=(kt == 0), stop=(kt == 15))

        def normalize(st):
            po_, b_, hh_, qc_ = st
            rs = op_pool.tile([1, T], f32r, tag="rs")
            with nc.allow_low_precision(reason="f32r reciprocal for softmax denom"):
                nc.vector.reciprocal(out=rs[:], in_=po_[64:65, :])
            pb2 = ps_m.tile([64, T], f32, tag="pb2")
            nc.tensor.matmul(pb2[:], ones64[:], rs[:], start=True, stop=True)
            rb = op_pool.tile([64, T], bf16, tag="rb")
            nc.vector.tensor_copy(out=rb[:], in_=pb2[:])
            ot = op_pool.tile([64, T], bf16, tag="ot")
            nc.vector.tensor_tensor(ot[:], po_[0:64, :], rb[:], ALU.mult)
            nc.sync.dma_start(out=a2a2_in[b_ * 4 + qc_, hh_ * 64:hh_ * 64 + 64, :],
                              in_=ot[:])

        for b in range(2):
            blk0 = 4 * b
            for hh in range(2):
                kts = hp_pool.tile([64, 4, T], bf16, tag="kts")
                nc.sync.dma_start(out=kts[:],
                                  in_=cc_kv_out[blk0:blk0 + 4, hh * 64:hh * 64 + 64, :]
                                  .transpose([1, 0, 2]))
                qts = hp_pool.tile([64, 4, T], bf16, tag="qts")
                nc.sync.dma_start(out=qts[:],
                                  in_=cc_q_out[blk0:blk0 + 4, hh * 64:hh * 64 + 64, :]
                                  .transpose([1, 0, 2]))
                vt = hp_pool.tile([128, 16, 65], bf16, tag="vt")
                nc.vector.memset(vt[:, :, 64:65], 1.0)
                for j in range(4):
                    vsec = cc_kv_out[blk0 + j, 128:256, :].rearrange("r t -> (r t)").rearrange(
                        "(a p d) -> p a d", p=128, d=128)
                    nc.sync.dma_start(out=vt[:, j * 4:(j + 1) * 4, 0:64],
                                      in_=vsec[:, :, hh * 64:hh * 64 + 64])
                delt = hp_pool.tile([128, 16], bf16, tag="delt")
                for j in range(4):
                    nc.sync.dma_start(out=delt[:, j * 4:(j + 1) * 4],
                                      in_=cc_kv_out[blk0 + j, 256 + hh, :]
                                      .rearrange("(a p) -> p a", p=128))
                deltf = hp_pool.tile([128, 16], f32, tag="deltf")
                nc.vector.tensor_copy(out=deltf[:], in_=delt[:])
                vts = hp_pool.tile([128, 16, 65], bf16, tag="vts")
                for kt in range(16):
                    nc.vector.tensor_scalar(vts[:, kt, :], vt[:, kt, :],
                                            deltf[:, kt:kt + 1], None, op0=ALU.mult)
                kflat = kts.rearrange("p a t -> p (a t)")
                for qc in range(4):
                    po = ps_o.tile([65, T], f32, tag="po")
                    for grp in range(8):
                        s2 = ps_s.tile([128, 2, T], f32, tag="s2")
                        for i2 in range(2):
                            kt = grp * 2 + i2
                            nc.tensor.matmul(s2[:, i2, :], kflat[:, kt * 128:(kt + 1) * 128],
                                             qts[:, qc, :], start=True, stop=True)
                        if prev is not None:
                            flush_av(prev)
                            prev = None
                        if grp == 1 and pend is not None:
                            normalize(pend)
                            pend = None
                        e2 = ep.tile([128, 2, T], bf16, tag="e2")
                        nc.scalar.activation(out=e2[:], in_=s2[:], func=AF.Exp,
                                             bias=0.0, scale=1.0)
                        prev = (po, e2, grp, vts)
                    pend = (po, b, hh, qc)
        flush_av(prev)
        prev = None
        normalize(pend)
        pend = None

    # ---- A2A #2 ----
    nc.gpsimd.collective_compute("AllToAll", ALU.bypass, replica_groups=GROUPS,
                                 ins=[a2a2_in[:]], outs=[a2a2_out[:]])

    # ---- Phase C: token-parallel out_proj ----
    with (
        tc.tile_pool(name="cw", bufs=2) as cw,
        tc.tile_pool(name="cin", bufs=1) as cin,
        tc.tile_pool(name="cev", bufs=3) as cev,
        tc.tile_pool(name="cdbg", bufs=2) as cdbg,
        tc.tile_pool(name="ps_c", bufs=4, space="PSUM") as ps_c,
    ):
        at = cin.tile([128, KT, T], bf16, tag="at")
        nc.sync.dma_start(out=at[:], in_=a2a2_out.rearrange("s (q p) t -> p (s q) t", p=128))
        if "attnT" in dbg:
            for kt in range(KT):
                _dbg_copy(nc, cdbg, dbg["attnT"][kt * 128:(kt + 1) * 128, :],
                          at[:, kt, :], (128, T))
        bcol2 = cin.tile([128, KT, 7], f32, tag="bcol2")
        nc.sync.dma_start(out=bcol2[:], in_=g("bcol_d")[:])
        for mt in range(KT):
            wt = cw.tile([128, KT, 128], bf16, tag="ow")
            nc.sync.dma_start(out=wt[:], in_=g("ow_d")[mt])
            p = ps_c.tile([128, T], f32, tag="pc")
            for kt in range(KT):
                nc.tensor.matmul(p[:], wt[:, kt, :], at[:, kt, :],
                                 start=(kt == 0), stop=(kt == KT - 1))
            ev = cev.tile([128, T], bf16, tag="cev")
            nc.vector.tensor_scalar_add(ev[:], p[:], bcol2[:, mt, 6:7])
            nc.sync.dma_start(out=yT[mt * 128:(mt + 1) * 128, :], in_=ev[:])


def make_inputs(full):
    """full: dict of original reference inputs -> list of 8 per-core in_maps."""
    import ml_dtypes
    bf = lambda a: np.ascontiguousarray(np.asarray(a, np.float32)).astype(ml_dtypes.bfloat16)
    f = lambda a: np.ascontiguousarray(np.asarray(a, dtype=np.float32))
    x = np.asarray(full["x"], dtype=np.float32)

    def tile_w(WT, nk):  # WT (nk*128, D) -> (KT, 128, nk, 128)
        return np.ascontiguousarray(
            WT.reshape(nk, 128, KT, 128).transpose(2, 1, 0, 3))

    def lin_w(w):        # torch (out, in) -> tiled W.T
        return tile_w(np.asarray(w, np.float32).T, KT)

    def conv_w(w):       # (D out, D in, 3) -> (KT mt, 128 p, 24 (tap,kt), 128 m)
        wt = np.asarray(w, np.float32).transpose(2, 1, 0)      # (tap, in, out)
        wt = wt.reshape(3, KT, 128, KT, 128)                   # tap, kt, p, mt, m
        return np.ascontiguousarray(wt.transpose(3, 2, 0, 1, 4).reshape(KT, 128, 24, 128))

    def proj_w(w):       # (D out, 2D in) -> (KT, 128, 16, 128)
        return tile_w(np.asarray(w, np.float32).T, 16)

    wv = np.asarray(full["Wv_w"], np.float32).T                # (in, out)
    wv_t = np.ascontiguousarray(
        wv.reshape(KT, 128, 2, 512).transpose(2, 1, 0, 3))     # (2, 128, KT, 512)

    bcol = np.stack([f(full[k]).reshape(KT, 128).T for k in
                     ["Wq_b", "Wk_b", "convq_b", "convk_b", "qproj_b", "kproj_b",
                      "out_b"]], axis=2)                        # (128, KT, 7)
    bbrd = f(full["Wv_b"]).reshape(1, D)

    perm = np.concatenate([gr * 128 + np.concatenate([np.arange(0, 128, 2),
                                                      np.arange(1, 128, 2)])
                           for gr in range(16)])
    tau1p = np.concatenate([np.asarray(full["tau1_w"], np.float32)[:, 0, :],
                            np.asarray(full["tau1_b"], np.float32)[:, None]], axis=1)[perm]
    del1p = np.concatenate([np.asarray(full["del1_w"], np.float32)[:, 0, :],
                            np.asarray(full["del1_b"], np.float32)[:, None]], axis=1)[perm]
    t2w = np.asarray(full["tau2_w"], np.float32)[:, :, 0].T[perm]  # (2048, 16)
    d2w = np.asarray(full["del2_w"], np.float32)[:, :, 0].T[perm]
    arr3 = lambda a: np.ascontiguousarray(a.reshape(16, 128, a.shape[-1]).transpose(1, 0, 2))

    common = {
        "wq": lin_w(full["Wq_w"]), "wk": lin_w(full["Wk_w"]), "wv": wv_t,
        "cq": conv_w(full["convq_w"]), "ck": conv_w(full["convk_w"]),
        "qp": proj_w(full["qproj_w"]), "kp": proj_w(full["kproj_w"]),
        "ow": lin_w(full["out_w"]),
        "bcol": bcol, "bbrd": bbrd,
        "tau1p": arr3(tau1p), "del1p": arr3(del1p),
        "t2w": arr3(t2w), "d2w": arr3(d2w),
        "t2b": f(full["tau2_b"]).reshape(16, 1), "d2b": f(full["del2_b"]).reshape(16, 1),
    }
    for k in ["wq", "wk", "wv", "cq", "ck", "qp", "kp", "ow", "bbrd", "t2w", "d2w"]:
        common[k] = bf(common[k])

    ins = []
    for c in range(NCORES):
        b, t0 = c // 4, (c % 4) * T
        xb = np.zeros((TH, D), np.float32)
        lo, hi = max(t0 - 1, 0), min(t0 + T + 1, L)
        xb[lo - (t0 - 1):hi - (t0 - 1)] = x[b, lo:hi]
        xrt = np.ascontiguousarray(xb.T.reshape(KT, 128, TH).transpose(1, 0, 2))
        m = dict(common)
        m["xr"] = bf(xrt)
        m["mlo"] = np.array([0.0 if t0 == 0 else 1.0], np.float32)
        m["mhi"] = np.array([0.0 if t0 + T == L else 1.0], np.float32)
        ins.append(m)
    return ins


def assemble(results):
    y = np.empty((B, L, D), np.float32)
    for c in range(NCORES):
        b, t0 = c // 4, (c % 4) * T
        y[b, t0:t0 + T] = np.asarray(results[c]["yT"], dtype=np.float32).T
    return y


def kernel(**inputs):
    """Takes the full unsharded reference inputs, returns the full (B, L, D) output."""
    from concourse.bass_utils import run_bass_kernel_spmd
    nc, _ = build()
    in_maps = make_inputs(inputs)
    res = run_bass_kernel_spmd(nc, in_maps, list(range(NCORES)))
    return assemble(res.results)
